# revision 31
# baseline (speedup 1.0000x reference)
"""Multi-head causal attention (B=4, T=2048, C=1024, H=16) on 8 trn2 cores.

Core = (batch b, head-half hg): 8 heads (4 head-pairs pj) per core. Phases
are interleaved per 512-token chunk so all engines stay busy: P1 (QKV
projection) for chunk nt, then P2 (causal attention for query chunk qt=nt,
scores kept transposed [key, query]; softmax denominators come from an
appended ones-column in V), then P3 (partial output projection for those
tokens). Causality is trimmed at 128-query granularity on diagonal key
blocks; only the single [128,128] diagonal tile needs a triangle mask.
Host sums the two half-projections per batch (proj bias folded into hg==0).
"""

import numpy as np
import ml_dtypes
import concourse.bass as bass
import concourse.mybir as mybir
import concourse.tile as tile
from concourse import bacc
from concourse.bass_utils import run_bass_kernel_spmd

B, T, C = 4, 2048, 1024
H, D = 16, 64
F32 = mybir.dt.float32
BF16 = mybir.dt.bfloat16
AFT = mybir.ActivationFunctionType

_CACHE = {}


def build():
    nc = bacc.Bacc(None, target_bir_lowering=False)
    xt_d = nc.dram_tensor("xt", [4096, 512], BF16, kind="ExternalInput")
    wq_d = nc.dram_tensor("wq", [C, 512], BF16, kind="ExternalInput")
    wk_d = nc.dram_tensor("wk", [C, 512], BF16, kind="ExternalInput")
    wv_d = nc.dram_tensor("wv", [C, 512], BF16, kind="ExternalInput")
    bqk_d = nc.dram_tensor("bqk", [128, 8], F32, kind="ExternalInput")
    bvb_d = nc.dram_tensor("bvb", [128, 512], F32, kind="ExternalInput")
    tri2_d = nc.dram_tensor("tri2", [128, 256], BF16, kind="ExternalInput")
    wp_d = nc.dram_tensor("wp", [512, C], BF16, kind="ExternalInput")
    wpb_d = nc.dram_tensor("wpb", [128, C], F32, kind="ExternalInput")
    out_d = nc.dram_tensor("out", [T, C], F32, kind="ExternalOutput")

    with nc.allow_low_precision(reason="bf16 matmul pipeline"):
        with tile.TileContext(nc) as tc:
            with (
                tc.tile_pool(name="const", bufs=1) as constp,
                tc.tile_pool(name="wpool", bufs=1) as wpool,
                tc.tile_pool(name="xpool", bufs=1) as xp,
                tc.tile_pool(name="qk", bufs=1) as qkp,
                tc.tile_pool(name="vpool", bufs=1) as vp,
                tc.tile_pool(name="ypool", bufs=1) as yp,
                tc.tile_pool(name="esb", bufs=6) as ep,
                tc.tile_pool(name="small", bufs=2) as smallp,
                tc.tile_pool(name="ps", bufs=2, space="PSUM") as psp,
            ):
                # ---- constants ----
                bqk_t = constp.tile([128, 8], F32, tag="bqk")
                nc.gpsimd.dma_start(bqk_t[:], bqk_d[:])
                bvb_t = constp.tile([128, 512], F32, tag="bvb")
                tri2_t = constp.tile([128, 256], BF16, tag="tri2")
                wpb_t = constp.tile([128, C], F32, tag="wpb")

                # ---- weights + x tiles (q weights and first x chunk first) ----
                wq_t = [wpool.tile([128, 512], BF16, tag=f"wq{c}", name=f"wq{c}") for c in range(8)]
                wk_t = [wpool.tile([128, 512], BF16, tag=f"wk{c}", name=f"wk{c}") for c in range(8)]
                wv_t = [wpool.tile([128, 512], BF16, tag=f"wv{c}", name=f"wv{c}") for c in range(8)]
                wp_t = [wpool.tile([128, C], BF16, tag=f"wp{c}", name=f"wp{c}") for c in range(4)]
                xt_t = [[xp.tile([128, 512], BF16, tag=f"x{c}_{nt}", name=f"x{c}_{nt}")
                         for nt in range(4)] for c in range(8)]
                def xrow(c, nt):
                    i = (c * 4 + nt) * 128
                    return xt_d[i:i + 128, :]

                engs = (nc.sync, nc.scalar, nc.gpsimd)
                xfers = []
                for c in range(8):
                    xfers.append((wq_t[c][:], wq_d[c * 128:(c + 1) * 128, :]))
                    xfers.append((xt_t[c][0][:], xrow(c, 0)))
                    xfers.append((wk_t[c][:], wk_d[c * 128:(c + 1) * 128, :]))
                for c in range(8):
                    xfers.append((wv_t[c][:], wv_d[c * 128:(c + 1) * 128, :]))
                xfers.append((bvb_t[:], bvb_d[:]))
                xfers.append((tri2_t[:], tri2_d[:]))
                for nt in range(1, 4):
                    for c in range(8):
                        xfers.append((xt_t[c][nt][:], xrow(c, nt)))
                for c4 in range(4):
                    xfers.append((wp_t[c4][:], wp_d[c4 * 128:(c4 + 1) * 128, :]))
                xfers.append((wpb_t[:], wpb_d[:]))
                for i, (dst, src_ap) in enumerate(xfers):
                    engs[i % 3].dma_start(dst, src_ap)

                # ---- SBUF result tiles ----
                qS = [[qkp.tile([128, 512], BF16, tag=f"q{pj}_{nt}", name=f"q{pj}_{nt}")
                       for nt in range(4)] for pj in range(4)]
                kS = [[qkp.tile([128, 512], BF16, tag=f"k{pj}_{nt}", name=f"k{pj}_{nt}")
                       for nt in range(4)] for pj in range(4)]
                vS = [vp.tile([128, 520], BF16, tag=f"v{tt}", name=f"v{tt}") for tt in range(16)]
                yS = [[yp.tile([128, 512], BF16, tag=f"y{pj}_{qt}", name=f"y{pj}_{qt}")
                       for qt in range(4)] for pj in range(4)]

                # ones columns of vS (softmax denominators ride along in PV)
                for tt in range(16):
                    vv = vS[tt][:].rearrange("p (h c) -> p h c", c=65)
                    nc.vector.memset(vv[:, :, 64:65], 1.0)

                def P1(nt):
                    # ---------------- P1: QKV projections for chunk nt ----------------
                    for ft in range(4):
                        q_ps = psp.tile([128, 512], F32, tag="mm")
                        for c in range(8):
                            nc.tensor.matmul(q_ps[:], wq_t[c][:, ft * 128:(ft + 1) * 128],
                                             xt_t[c][nt][:], start=(c == 0), stop=(c == 7))
                        nc.vector.tensor_scalar_add(qS[ft][nt][:], q_ps[:],
                                                    bqk_t[:, ft:ft + 1])
                        k_ps = psp.tile([128, 512], F32, tag="mm")
                        for c in range(8):
                            nc.tensor.matmul(k_ps[:], wk_t[c][:, ft * 128:(ft + 1) * 128],
                                             xt_t[c][nt][:], start=(c == 0), stop=(c == 7))
                        nc.vector.tensor_scalar_add(kS[ft][nt][:], k_ps[:],
                                                    bqk_t[:, 4 + ft:5 + ft])
                    for t2 in range(4):
                        tt = nt * 4 + t2
                        v_ps = psp.tile([128, 512], F32, tag="mm")
                        for c in range(8):
                            nc.tensor.matmul(v_ps[:], xt_t[c][nt][:, t2 * 128:(t2 + 1) * 128],
                                             wv_t[c][:], start=(c == 0), stop=(c == 7))
                        vv = vS[tt][:].rearrange("p (h c) -> p h c", c=65)
                        nc.vector.tensor_add(vv[:, :, 0:64],
                                             v_ps[:].rearrange("p (h c) -> p h c", c=64),
                                             bvb_t[:].rearrange("p (h c) -> p h c", c=64))

                def P2(qt, fills=None, sc_fills=None):
                    # ---------------- P2: attention for query chunk qt ----------------
                    nblk = 4 * qt + 4
                    for pj in range(4):
                        y_ps = [psp.tile([65, 512], F32, tag=f"yps{h}", bufs=1, name=f"yps{h}")
                                for h in range(2)]
                        for sc in range(nblk):
                            r = sc - 4 * qt
                            qlo = 128 * r if r > 0 else 0
                            ntk, kk = sc // 4, sc % 4
                            s_ps = psp.tile([128, 1024], F32, tag="sps")
                            nc.tensor.matmul(s_ps[:, qlo:512],
                                             kS[pj][ntk][0:64, kk * 128:(kk + 1) * 128],
                                             qS[pj][qt][0:64, qlo:512],
                                             start=True, stop=True, tile_position=(0, 0))
                            nc.tensor.matmul(s_ps[:, 512 + qlo:1024],
                                             kS[pj][ntk][64:128, kk * 128:(kk + 1) * 128],
                                             qS[pj][qt][64:128, qlo:512],
                                             start=True, stop=True, tile_position=(64, 0))
                            e_t = ep.tile([128, 1024], BF16, tag="e")
                            sv = s_ps[:].rearrange("p (g n) -> p g n", g=2)[:, :, qlo:512]
                            ev = e_t[:].rearrange("p (g n) -> p g n", g=2)[:, :, qlo:512]
                            nc.scalar.activation(ev, sv, AFT.Exp, scale=0.125)
                            if r >= 0:
                                em = e_t[:].rearrange("p (g n) -> p g n", g=2)[:, :, qlo:qlo + 128]
                                nc.vector.tensor_mul(
                                    em, em, tri2_t[:].rearrange("p (g n) -> p g n", g=2))
                            for h in range(2):
                                hc = 130 * pj + 65 * h
                                nc.tensor.matmul(y_ps[h][:, qlo:512],
                                                 vS[sc][:, hc:hc + 65],
                                                 e_t[:, 512 * h + qlo:512 * h + 512],
                                                 start=(sc == 0), stop=(sc == nblk - 1))
                            if sc_fills:
                                f = sc_fills.pop((pj, sc), None)
                                if f is not None:
                                    f()
                        if qt == 3 and pj == 3:
                            o_ps7 = psp.tile([128, 512], F32, tag="mm")
                            for i2, pj2 in enumerate((0, 1, 2)):
                                nc.tensor.matmul(o_ps7[:],
                                                 yS[pj2][3][:, 3 * 128:4 * 128],
                                                 wp_t[pj2][:, 512:1024],
                                                 start=(i2 == 0), stop=(i2 == 2))
                            nc.vector.tensor_add(parts[7][:], o_ps7[:],
                                                 wpb_t[:, 512:1024])
                        for h in range(2):
                            y_sb = smallp.tile([65, 512], F32, tag="ysb", bufs=6)
                            if qt == 3 and pj == 3:
                                nc.scalar.activation(y_sb[:], y_ps[h][:], AFT.Copy)
                            else:
                                nc.vector.tensor_copy(y_sb[:], y_ps[h][:])
                            den = smallp.tile([1, 512], F32, tag="den", bufs=4)
                            nc.sync.dma_start(den[:], y_sb[64:65, :])
                            rec = smallp.tile([1, 512], F32, tag="rec", bufs=4)
                            nc.vector.reciprocal_approx_fast(rec[:], den[:])
                            rb = smallp.tile([64, 512], F32, tag="rb", bufs=4)
                            nc.gpsimd.partition_broadcast(rb[:], rec[:])
                            nc.vector.tensor_mul(yS[pj][qt][64 * h:64 * h + 64, :],
                                                 y_sb[0:64, :], rb[:])
                        if fills:
                            for f in fills.get(pj, ()):
                                f()

                def P3_group(qt, t2, of):
                    tt = 4 * qt + t2
                    o_ps = psp.tile([128, 512], F32, tag="mm")
                    for i, pj in enumerate((3, 0, 1, 2)):
                        nc.tensor.matmul(o_ps[:],
                                         yS[pj][qt][:, t2 * 128:(t2 + 1) * 128],
                                         wp_t[pj][:, of * 512:(of + 1) * 512],
                                         start=(i == 0), stop=(i == 3))
                    o_t = smallp.tile([128, 512], F32, tag="osb", bufs=3)
                    nc.vector.tensor_add(o_t[:], o_ps[:],
                                         wpb_t[:, of * 512:(of + 1) * 512])
                    nc.sync.dma_start(out_d[tt * 128:(tt + 1) * 128,
                                            of * 512:(of + 1) * 512], o_t[:])

                def P3_fills(qt):
                    gs = [(t2, of) for t2 in range(4) for of in range(2)]
                    return {pj: [lambda t2=t2, of=of: P3_group(qt, t2, of)
                                 for (t2, of) in gs[2 * pj:2 * pj + 2]]
                            for pj in range(4)}

                # pass-A/pass-B split of the last projection: partials over
                # head-pairs 0-2 (+bias) computed during P2(3); only one
                # matmul per group remains after the last normalization.
                parts = [smallp.tile([128, 512], F32, tag=f"part{g}",
                                     bufs=1, name=f"part{g}") for g in range(8)]

                def P3a_group(qt, t2, of):
                    o_ps = psp.tile([128, 512], F32, tag="mm")
                    for i, pj in enumerate((0, 1, 2)):
                        nc.tensor.matmul(o_ps[:],
                                         yS[pj][qt][:, t2 * 128:(t2 + 1) * 128],
                                         wp_t[pj][:, of * 512:(of + 1) * 512],
                                         start=(i == 0), stop=(i == 2))
                    nc.vector.tensor_add(parts[t2 * 2 + of][:], o_ps[:],
                                         wpb_t[:, of * 512:(of + 1) * 512])

                def P3b(qt):
                    for t2 in range(4):
                        tt = 4 * qt + t2
                        for of in range(2):
                            o_ps = psp.tile([128, 512], F32, tag="mm")
                            nc.tensor.matmul(o_ps[:],
                                             yS[3][qt][:, t2 * 128:(t2 + 1) * 128],
                                             wp_t[3][:, of * 512:(of + 1) * 512],
                                             start=True, stop=True)
                            o_t = smallp.tile([128, 512], F32, tag="osb", bufs=3)
                            nc.vector.tensor_add(o_t[:], o_ps[:],
                                                 parts[t2 * 2 + of][:])
                            nc.sync.dma_start(out_d[tt * 128:(tt + 1) * 128,
                                                    of * 512:(of + 1) * 512], o_t[:])

                # P1(nt+1) is ordered before P2(nt+1); P3(qt-1) groups are
                # spread across P2(qt)'s head-pair boundaries to fill its
                # exp-bound stretches; P3(3) is split so only one matmul per
                # group trails the last normalization.
                P1(0)
                P2(0)
                P1(1)
                P2(1, fills=P3_fills(0))
                P1(2)
                P2(2, fills=P3_fills(1))
                P1(3)
                f2 = P3_fills(2)
                scf = {(3, 2 + 3 * g): (lambda g=g: P3a_group(3, g // 2, g % 2))
                       for g in range(5)}
                f3 = {2: f2[2] + f2[3] + [lambda: P3a_group(3, 2, 1),
                                          lambda: P3a_group(3, 3, 0)]}
                P2(3, fills={**{pj: f2[pj] for pj in (0, 1)}, **f3},
                   sc_fills=scf)
                P3b(3)

    if not nc.is_finalized():
        nc.finalize()
    return nc


def _get_nc():
    if "nc" not in _CACHE:
        _CACHE["nc"] = build()
    return _CACHE["nc"]


def _tri2():
    k = np.arange(128)[:, None]
    q = np.arange(128)[None, :]
    t = np.where(k <= q, 1.0, 0.0).astype(ml_dtypes.bfloat16)
    return np.concatenate([t, t], axis=1)


def kernel(x, w_attn, b_attn, w_proj, b_proj, _trace=False, _trace_kwargs=None):
    x = np.asarray(x, dtype=np.float32)
    w_attn = np.asarray(w_attn, dtype=np.float32)
    b_attn = np.asarray(b_attn, dtype=np.float32)
    w_proj = np.asarray(w_proj, dtype=np.float32)
    b_proj = np.asarray(b_proj, dtype=np.float32)

    bf = ml_dtypes.bfloat16
    tri2 = _tri2()
    in_maps = []
    for core in range(8):
        b, hg = core // 2, core % 2
        cs = hg * 512
        bq = b_attn[cs:cs + 512]
        bk = b_attn[C + cs:C + cs + 512]
        bv = b_attn[2 * C + cs:2 * C + cs + 512]
        bqk = np.concatenate([bq.reshape(4, 128).T, bk.reshape(4, 128).T],
                             axis=1).astype(np.float32)
        wpb = b_proj if hg == 0 else np.zeros_like(b_proj)
        in_maps.append({
            "xt": np.ascontiguousarray(
                x[b].T.reshape(8, 128, 4, 512).transpose(0, 2, 1, 3)
                .reshape(4096, 512)).astype(bf),
            "wq": np.ascontiguousarray(w_attn[:, cs:cs + 512]).astype(bf),
            "wk": np.ascontiguousarray(w_attn[:, C + cs:C + cs + 512]).astype(bf),
            "wv": np.ascontiguousarray(w_attn[:, 2 * C + cs:2 * C + cs + 512]).astype(bf),
            "bqk": bqk,
            "bvb": np.ascontiguousarray(
                np.broadcast_to(bv[None, :], (128, 512))).astype(np.float32),
            "tri2": tri2,
            "wp": np.ascontiguousarray(w_proj[cs:cs + 512, :]).astype(bf),
            "wpb": np.ascontiguousarray(
                np.broadcast_to(wpb[None, :], (128, C))).astype(np.float32),
        })

    kw = {}
    if _trace:
        kw["trace"] = True
        if _trace_kwargs:
            kw.update(_trace_kwargs)
    res = run_bass_kernel_spmd(_get_nc(), in_maps, list(range(8)), **kw)
    _CACHE["last_results"] = res
    outs = [res.results[c]["out"] for c in range(8)]
    y = np.stack([outs[2 * b] + outs[2 * b + 1] for b in range(B)])
    return y.astype(np.float32)


# revision 32
# speedup vs baseline: 1.1840x; 1.1840x over previous
"""Multi-head causal attention (B=4, T=2048, C=1024, H=16) on 8 trn2 cores.

Core = (batch b, head-half hg): 8 heads (4 head-pairs pj) per core. Phases
are interleaved per 512-token chunk so all engines stay busy: P1 (QKV
projection) for chunk nt, then P2 (causal attention for query chunk qt=nt,
scores kept transposed [key, query]; softmax denominators come from an
appended ones-column in V), then P3 (partial output projection for those
tokens). Causality is trimmed at 128-query granularity on diagonal key
blocks; only the single [128,128] diagonal tile needs a triangle mask.
Host sums the two half-projections per batch (proj bias folded into hg==0).
"""

import numpy as np
import ml_dtypes
import concourse.bass as bass
import concourse.mybir as mybir
import concourse.tile as tile
from concourse import bacc
from concourse.bass_utils import run_bass_kernel_spmd

B, T, C = 4, 2048, 1024
H, D = 16, 64
F32 = mybir.dt.float32
BF16 = mybir.dt.bfloat16
AFT = mybir.ActivationFunctionType

_CACHE = {}


def build():
    nc = bacc.Bacc(None, target_bir_lowering=False)
    xt_d = nc.dram_tensor("xt", [4096, 512], BF16, kind="ExternalInput")
    wq_d = nc.dram_tensor("wq", [C, 512], BF16, kind="ExternalInput")
    wk_d = nc.dram_tensor("wk", [C, 512], BF16, kind="ExternalInput")
    wv_d = nc.dram_tensor("wv", [C, 512], BF16, kind="ExternalInput")
    bqk_d = nc.dram_tensor("bqk", [128, 8], F32, kind="ExternalInput")
    bvb_d = nc.dram_tensor("bvb", [128, 512], F32, kind="ExternalInput")
    tri2_d = nc.dram_tensor("tri2", [128, 256], BF16, kind="ExternalInput")
    wp_d = nc.dram_tensor("wp", [512, C], BF16, kind="ExternalInput")
    wpb_d = nc.dram_tensor("wpb", [128, C], F32, kind="ExternalInput")
    out_d = nc.dram_tensor("out", [T, C], F32, kind="ExternalOutput")

    with nc.allow_low_precision(reason="bf16 matmul pipeline"):
        with tile.TileContext(nc) as tc:
            with (
                tc.tile_pool(name="const", bufs=1) as constp,
                tc.tile_pool(name="wpool", bufs=1) as wpool,
                tc.tile_pool(name="xpool", bufs=1) as xp,
                tc.tile_pool(name="qk", bufs=1) as qkp,
                tc.tile_pool(name="vpool", bufs=1) as vp,
                tc.tile_pool(name="ypool", bufs=1) as yp,
                tc.tile_pool(name="esb", bufs=6) as ep,
                tc.tile_pool(name="small", bufs=2) as smallp,
                tc.tile_pool(name="ps", bufs=2, space="PSUM") as psp,
            ):
                # ---- constants ----
                bqk_t = constp.tile([128, 8], F32, tag="bqk")
                nc.gpsimd.dma_start(bqk_t[:], bqk_d[:])
                bvb_t = constp.tile([128, 512], F32, tag="bvb")
                tri2_t = constp.tile([128, 256], BF16, tag="tri2")
                wpb_t = constp.tile([128, C], F32, tag="wpb")

                # ---- weights + x tiles (q weights and first x chunk first) ----
                wq_t = [wpool.tile([128, 512], BF16, tag=f"wq{c}", name=f"wq{c}") for c in range(8)]
                wk_t = [wpool.tile([128, 512], BF16, tag=f"wk{c}", name=f"wk{c}") for c in range(8)]
                wv_t = [wpool.tile([128, 512], BF16, tag=f"wv{c}", name=f"wv{c}") for c in range(8)]
                wp_t = [wpool.tile([128, C], BF16, tag=f"wp{c}", name=f"wp{c}") for c in range(4)]
                xt_t = [[xp.tile([128, 512], BF16, tag=f"x{c}_{nt}", name=f"x{c}_{nt}")
                         for nt in range(4)] for c in range(8)]
                def xrow(c, nt):
                    i = (c * 4 + nt) * 128
                    return xt_d[i:i + 128, :]

                engs = (nc.sync, nc.scalar, nc.gpsimd)
                xfers = []
                for c in range(8):
                    xfers.append((wq_t[c][:], wq_d[c * 128:(c + 1) * 128, :]))
                    xfers.append((xt_t[c][0][:], xrow(c, 0)))
                    xfers.append((wk_t[c][:], wk_d[c * 128:(c + 1) * 128, :]))
                for c in range(8):
                    xfers.append((wv_t[c][:], wv_d[c * 128:(c + 1) * 128, :]))
                xfers.append((bvb_t[:], bvb_d[:]))
                xfers.append((tri2_t[:], tri2_d[:]))
                for nt in range(1, 4):
                    for c in range(8):
                        xfers.append((xt_t[c][nt][:], xrow(c, nt)))
                for c4 in range(4):
                    xfers.append((wp_t[c4][:], wp_d[c4 * 128:(c4 + 1) * 128, :]))
                xfers.append((wpb_t[:], wpb_d[:]))
                for i, (dst, src_ap) in enumerate(xfers):
                    engs[i % 3].dma_start(dst, src_ap)

                # ---- SBUF result tiles ----
                qS = [[qkp.tile([128, 512], BF16, tag=f"q{pj}_{nt}", name=f"q{pj}_{nt}")
                       for nt in range(4)] for pj in range(4)]
                kS = [[qkp.tile([128, 512], BF16, tag=f"k{pj}_{nt}", name=f"k{pj}_{nt}")
                       for nt in range(4)] for pj in range(4)]
                vS = [vp.tile([128, 520], BF16, tag=f"v{tt}", name=f"v{tt}") for tt in range(16)]
                yS = [[yp.tile([128, 512], BF16, tag=f"y{pj}_{qt}", name=f"y{pj}_{qt}")
                       for qt in range(4)] for pj in range(4)]

                # ones columns of vS (softmax denominators ride along in PV)
                for tt in range(16):
                    vv = vS[tt][:].rearrange("p (h c) -> p h c", c=65)
                    nc.vector.memset(vv[:, :, 64:65], 1.0)

                def P1(nt):
                    # ---------------- P1: QKV projections for chunk nt ----------------
                    for ft in range(4):
                        q_ps = psp.tile([128, 512], F32, tag="mm")
                        for c in range(8):
                            nc.tensor.matmul(q_ps[:], wq_t[c][:, ft * 128:(ft + 1) * 128],
                                             xt_t[c][nt][:], start=(c == 0), stop=(c == 7))
                        nc.vector.tensor_scalar_add(qS[ft][nt][:], q_ps[:],
                                                    bqk_t[:, ft:ft + 1])
                        k_ps = psp.tile([128, 512], F32, tag="mm")
                        for c in range(8):
                            nc.tensor.matmul(k_ps[:], wk_t[c][:, ft * 128:(ft + 1) * 128],
                                             xt_t[c][nt][:], start=(c == 0), stop=(c == 7))
                        nc.vector.tensor_scalar_add(kS[ft][nt][:], k_ps[:],
                                                    bqk_t[:, 4 + ft:5 + ft])
                    for t2 in range(4):
                        tt = nt * 4 + t2
                        v_ps = psp.tile([128, 512], F32, tag="mm")
                        for c in range(8):
                            nc.tensor.matmul(v_ps[:], xt_t[c][nt][:, t2 * 128:(t2 + 1) * 128],
                                             wv_t[c][:], start=(c == 0), stop=(c == 7))
                        vv = vS[tt][:].rearrange("p (h c) -> p h c", c=65)
                        nc.vector.tensor_add(vv[:, :, 0:64],
                                             v_ps[:].rearrange("p (h c) -> p h c", c=64),
                                             bvb_t[:].rearrange("p (h c) -> p h c", c=64))

                def P2(qt, fills=None, sc_fills=None):
                    # ---------------- P2: attention for query chunk qt ----------------
                    nblk = 4 * qt + 4
                    for pj in range(4):
                        y_ps = [psp.tile([65, 512], F32, tag=f"yps{h}", bufs=1, name=f"yps{h}")
                                for h in range(2)]
                        for sc in range(nblk):
                            r = sc - 4 * qt
                            qlo = 128 * r if r > 0 else 0
                            ntk, kk = sc // 4, sc % 4
                            s_ps = psp.tile([128, 1024], F32, tag="sps")
                            nc.tensor.matmul(s_ps[:, qlo:512],
                                             kS[pj][ntk][0:64, kk * 128:(kk + 1) * 128],
                                             qS[pj][qt][0:64, qlo:512],
                                             start=True, stop=True, tile_position=(0, 0))
                            nc.tensor.matmul(s_ps[:, 512 + qlo:1024],
                                             kS[pj][ntk][64:128, kk * 128:(kk + 1) * 128],
                                             qS[pj][qt][64:128, qlo:512],
                                             start=True, stop=True, tile_position=(64, 0))
                            e_t = ep.tile([128, 1024], BF16, tag="e")
                            sv = s_ps[:].rearrange("p (g n) -> p g n", g=2)[:, :, qlo:512]
                            ev = e_t[:].rearrange("p (g n) -> p g n", g=2)[:, :, qlo:512]
                            nc.scalar.activation(ev, sv, AFT.Exp, scale=0.125)
                            if r >= 0:
                                em = e_t[:].rearrange("p (g n) -> p g n", g=2)[:, :, qlo:qlo + 128]
                                nc.vector.tensor_mul(
                                    em, em, tri2_t[:].rearrange("p (g n) -> p g n", g=2))
                            for h in range(2):
                                hc = 130 * pj + 65 * h
                                nc.tensor.matmul(y_ps[h][:, qlo:512],
                                                 vS[sc][:, hc:hc + 65],
                                                 e_t[:, 512 * h + qlo:512 * h + 512],
                                                 start=(sc == 0), stop=(sc == nblk - 1))
                            if sc_fills:
                                f = sc_fills.pop((pj, sc), None)
                                if f is not None:
                                    f()
                        for h in range(2):
                            y_sb = smallp.tile([65, 512], F32, tag="ysb", bufs=6)
                            nc.vector.tensor_copy(y_sb[:], y_ps[h][:])
                            den = smallp.tile([1, 512], F32, tag="den", bufs=4)
                            nc.sync.dma_start(den[:], y_sb[64:65, :])
                            rec = smallp.tile([1, 512], F32, tag="rec", bufs=4)
                            nc.vector.reciprocal_approx_fast(rec[:], den[:])
                            rb = smallp.tile([64, 512], F32, tag="rb", bufs=4)
                            nc.gpsimd.partition_broadcast(rb[:], rec[:])
                            nc.vector.tensor_mul(yS[pj][qt][64 * h:64 * h + 64, :],
                                                 y_sb[0:64, :], rb[:])
                        if fills:
                            for f in fills.get(pj, ()):
                                f()

                def P3_group(qt, t2, of):
                    tt = 4 * qt + t2
                    o_ps = psp.tile([128, 512], F32, tag="mm")
                    for i, pj in enumerate((3, 0, 1, 2)):
                        nc.tensor.matmul(o_ps[:],
                                         yS[pj][qt][:, t2 * 128:(t2 + 1) * 128],
                                         wp_t[pj][:, of * 512:(of + 1) * 512],
                                         start=(i == 0), stop=(i == 3))
                    o_t = smallp.tile([128, 512], F32, tag="osb", bufs=3)
                    nc.vector.tensor_add(o_t[:], o_ps[:],
                                         wpb_t[:, of * 512:(of + 1) * 512])
                    nc.sync.dma_start(out_d[tt * 128:(tt + 1) * 128,
                                            of * 512:(of + 1) * 512], o_t[:])

                def P3_fills(qt):
                    gs = [(t2, of) for t2 in range(4) for of in range(2)]
                    return {pj: [lambda t2=t2, of=of: P3_group(qt, t2, of)
                                 for (t2, of) in gs[2 * pj:2 * pj + 2]]
                            for pj in range(4)}

                # pass-A/pass-B split of the last projection: partials over
                # head-pairs 0-2 (+bias) computed during P2(3); only one
                # matmul per group remains after the last normalization.
                parts = [smallp.tile([128, 512], F32, tag=f"part{g}",
                                     bufs=1, name=f"part{g}") for g in range(8)]

                def P3a_group(qt, t2, of):
                    o_ps = psp.tile([128, 512], F32, tag="mm")
                    for i, pj in enumerate((0, 1, 2)):
                        nc.tensor.matmul(o_ps[:],
                                         yS[pj][qt][:, t2 * 128:(t2 + 1) * 128],
                                         wp_t[pj][:, of * 512:(of + 1) * 512],
                                         start=(i == 0), stop=(i == 2))
                    nc.vector.tensor_add(parts[t2 * 2 + of][:], o_ps[:],
                                         wpb_t[:, of * 512:(of + 1) * 512])

                def P3b(qt):
                    for t2 in range(4):
                        tt = 4 * qt + t2
                        for of in range(2):
                            o_ps = psp.tile([128, 512], F32, tag="mm")
                            nc.tensor.matmul(o_ps[:],
                                             yS[3][qt][:, t2 * 128:(t2 + 1) * 128],
                                             wp_t[3][:, of * 512:(of + 1) * 512],
                                             start=True, stop=True)
                            o_t = smallp.tile([128, 512], F32, tag="osb", bufs=3)
                            nc.vector.tensor_add(o_t[:], o_ps[:],
                                                 parts[t2 * 2 + of][:])
                            nc.sync.dma_start(out_d[tt * 128:(tt + 1) * 128,
                                                    of * 512:(of + 1) * 512], o_t[:])

                # P1(nt+1) is ordered before P2(nt+1); P3(qt-1) groups are
                # spread across P2(qt)'s head-pair boundaries to fill its
                # exp-bound stretches; P3(3) is split so only one matmul per
                # group trails the last normalization.
                P1(0)
                P2(0)
                P1(1)
                P2(1, fills=P3_fills(0))
                P1(2)
                P2(2, fills=P3_fills(1))
                P1(3)
                f2 = P3_fills(2)
                scf = {(3, 2 + 3 * g): (lambda g=g: P3a_group(3, g // 2, g % 2))
                       for g in range(5)}
                f3 = {2: f2[2] + f2[3] + [lambda: P3a_group(3, 2, 1),
                                          lambda: P3a_group(3, 3, 0),
                                          lambda: P3a_group(3, 3, 1)]}
                P2(3, fills={**{pj: f2[pj] for pj in (0, 1)}, **f3},
                   sc_fills=scf)
                P3b(3)

    if not nc.is_finalized():
        nc.finalize()
    return nc


def _get_nc():
    if "nc" not in _CACHE:
        _CACHE["nc"] = build()
    return _CACHE["nc"]


def _tri2():
    k = np.arange(128)[:, None]
    q = np.arange(128)[None, :]
    t = np.where(k <= q, 1.0, 0.0).astype(ml_dtypes.bfloat16)
    return np.concatenate([t, t], axis=1)


def kernel(x, w_attn, b_attn, w_proj, b_proj, _trace=False, _trace_kwargs=None):
    x = np.asarray(x, dtype=np.float32)
    w_attn = np.asarray(w_attn, dtype=np.float32)
    b_attn = np.asarray(b_attn, dtype=np.float32)
    w_proj = np.asarray(w_proj, dtype=np.float32)
    b_proj = np.asarray(b_proj, dtype=np.float32)

    bf = ml_dtypes.bfloat16
    tri2 = _tri2()
    in_maps = []
    for core in range(8):
        b, hg = core // 2, core % 2
        cs = hg * 512
        bq = b_attn[cs:cs + 512]
        bk = b_attn[C + cs:C + cs + 512]
        bv = b_attn[2 * C + cs:2 * C + cs + 512]
        bqk = np.concatenate([bq.reshape(4, 128).T, bk.reshape(4, 128).T],
                             axis=1).astype(np.float32)
        wpb = b_proj if hg == 0 else np.zeros_like(b_proj)
        in_maps.append({
            "xt": np.ascontiguousarray(
                x[b].T.reshape(8, 128, 4, 512).transpose(0, 2, 1, 3)
                .reshape(4096, 512)).astype(bf),
            "wq": np.ascontiguousarray(w_attn[:, cs:cs + 512]).astype(bf),
            "wk": np.ascontiguousarray(w_attn[:, C + cs:C + cs + 512]).astype(bf),
            "wv": np.ascontiguousarray(w_attn[:, 2 * C + cs:2 * C + cs + 512]).astype(bf),
            "bqk": bqk,
            "bvb": np.ascontiguousarray(
                np.broadcast_to(bv[None, :], (128, 512))).astype(np.float32),
            "tri2": tri2,
            "wp": np.ascontiguousarray(w_proj[cs:cs + 512, :]).astype(bf),
            "wpb": np.ascontiguousarray(
                np.broadcast_to(wpb[None, :], (128, C))).astype(np.float32),
        })

    kw = {}
    if _trace:
        kw["trace"] = True
        if _trace_kwargs:
            kw.update(_trace_kwargs)
    res = run_bass_kernel_spmd(_get_nc(), in_maps, list(range(8)), **kw)
    _CACHE["last_results"] = res
    outs = [res.results[c]["out"] for c in range(8)]
    y = np.stack([outs[2 * b] + outs[2 * b + 1] for b in range(B)])
    return y.astype(np.float32)


# revision 33
# speedup vs baseline: 1.1928x; 1.0075x over previous
"""Multi-head causal attention (B=4, T=2048, C=1024, H=16) on 8 trn2 cores.

Core = (batch b, head-half hg): 8 heads (4 head-pairs pj) per core. Phases
are interleaved per 512-token chunk so all engines stay busy: P1 (QKV
projection) for chunk nt, then P2 (causal attention for query chunk qt=nt,
scores kept transposed [key, query]; softmax denominators come from an
appended ones-column in V), then P3 (partial output projection for those
tokens). Causality is trimmed at 128-query granularity on diagonal key
blocks; only the single [128,128] diagonal tile needs a triangle mask.
Host sums the two half-projections per batch (proj bias folded into hg==0).
"""

import numpy as np
import ml_dtypes
import concourse.bass as bass
import concourse.mybir as mybir
import concourse.tile as tile
from concourse import bacc
from concourse.bass_utils import run_bass_kernel_spmd

B, T, C = 4, 2048, 1024
H, D = 16, 64
F32 = mybir.dt.float32
BF16 = mybir.dt.bfloat16
AFT = mybir.ActivationFunctionType

_CACHE = {}


def build():
    nc = bacc.Bacc(None, target_bir_lowering=False)
    xt_d = nc.dram_tensor("xt", [4096, 512], BF16, kind="ExternalInput")
    wq_d = nc.dram_tensor("wq", [C, 512], BF16, kind="ExternalInput")
    wk_d = nc.dram_tensor("wk", [C, 512], BF16, kind="ExternalInput")
    wv_d = nc.dram_tensor("wv", [C, 512], BF16, kind="ExternalInput")
    bqk_d = nc.dram_tensor("bqk", [128, 8], F32, kind="ExternalInput")
    bvb_d = nc.dram_tensor("bvb", [128, 512], F32, kind="ExternalInput")
    tri2_d = nc.dram_tensor("tri2", [128, 256], BF16, kind="ExternalInput")
    wp_d = nc.dram_tensor("wp", [512, C], BF16, kind="ExternalInput")
    wpb_d = nc.dram_tensor("wpb", [128, C], F32, kind="ExternalInput")
    out_d = nc.dram_tensor("out", [T, C], F32, kind="ExternalOutput")

    with nc.allow_low_precision(reason="bf16 matmul pipeline"):
        with tile.TileContext(nc) as tc:
            with (
                tc.tile_pool(name="const", bufs=1) as constp,
                tc.tile_pool(name="wpool", bufs=1) as wpool,
                tc.tile_pool(name="xpool", bufs=1) as xp,
                tc.tile_pool(name="qk", bufs=1) as qkp,
                tc.tile_pool(name="vpool", bufs=1) as vp,
                tc.tile_pool(name="ypool", bufs=1) as yp,
                tc.tile_pool(name="esb", bufs=6) as ep,
                tc.tile_pool(name="small", bufs=2) as smallp,
                tc.tile_pool(name="ps", bufs=2, space="PSUM") as psp,
            ):
                # ---- constants ----
                bqk_t = constp.tile([128, 8], F32, tag="bqk")
                nc.gpsimd.dma_start(bqk_t[:], bqk_d[:])
                bvb_t = constp.tile([128, 512], F32, tag="bvb")
                tri2_t = constp.tile([128, 256], BF16, tag="tri2")
                wpb_t = constp.tile([128, C], F32, tag="wpb")

                # ---- weights + x tiles (q weights and first x chunk first) ----
                wq_t = [wpool.tile([128, 512], BF16, tag=f"wq{c}", name=f"wq{c}") for c in range(8)]
                wk_t = [wpool.tile([128, 512], BF16, tag=f"wk{c}", name=f"wk{c}") for c in range(8)]
                wv_t = [wpool.tile([128, 512], BF16, tag=f"wv{c}", name=f"wv{c}") for c in range(8)]
                wp_t = [wpool.tile([128, C], BF16, tag=f"wp{c}", name=f"wp{c}") for c in range(4)]
                xt_t = [[xp.tile([128, 512], BF16, tag=f"x{c}_{nt}", name=f"x{c}_{nt}")
                         for nt in range(4)] for c in range(8)]
                def xrow(c, nt):
                    i = (c * 4 + nt) * 128
                    return xt_d[i:i + 128, :]

                engs = (nc.sync, nc.scalar, nc.gpsimd)
                xfers = []
                for c in range(8):
                    xfers.append((wq_t[c][:], wq_d[c * 128:(c + 1) * 128, :]))
                    xfers.append((xt_t[c][0][:], xrow(c, 0)))
                    xfers.append((wk_t[c][:], wk_d[c * 128:(c + 1) * 128, :]))
                for c in range(8):
                    xfers.append((wv_t[c][:], wv_d[c * 128:(c + 1) * 128, :]))
                xfers.append((bvb_t[:], bvb_d[:]))
                xfers.append((tri2_t[:], tri2_d[:]))
                for nt in range(1, 4):
                    for c in range(8):
                        xfers.append((xt_t[c][nt][:], xrow(c, nt)))
                for c4 in range(4):
                    xfers.append((wp_t[c4][:], wp_d[c4 * 128:(c4 + 1) * 128, :]))
                xfers.append((wpb_t[:], wpb_d[:]))
                for i, (dst, src_ap) in enumerate(xfers):
                    engs[i % 3].dma_start(dst, src_ap)

                # ---- SBUF result tiles ----
                qS = [[qkp.tile([128, 512], BF16, tag=f"q{pj}_{nt}", name=f"q{pj}_{nt}")
                       for nt in range(4)] for pj in range(4)]
                kS = [[qkp.tile([128, 512], BF16, tag=f"k{pj}_{nt}", name=f"k{pj}_{nt}")
                       for nt in range(4)] for pj in range(4)]
                vS = [vp.tile([128, 520], BF16, tag=f"v{tt}", name=f"v{tt}") for tt in range(16)]
                yS = [[yp.tile([128, 512], BF16, tag=f"y{pj}_{qt}", name=f"y{pj}_{qt}")
                       for qt in range(4)] for pj in range(4)]

                # ones columns of vS (softmax denominators ride along in PV)
                for tt in range(16):
                    vv = vS[tt][:].rearrange("p (h c) -> p h c", c=65)
                    nc.vector.memset(vv[:, :, 64:65], 1.0)

                def P1(nt):
                    # ---------------- P1: QKV projections for chunk nt ----------------
                    for ft in range(4):
                        q_ps = psp.tile([128, 512], F32, tag="mm")
                        for c in range(8):
                            nc.tensor.matmul(q_ps[:], wq_t[c][:, ft * 128:(ft + 1) * 128],
                                             xt_t[c][nt][:], start=(c == 0), stop=(c == 7))
                        nc.vector.tensor_scalar_add(qS[ft][nt][:], q_ps[:],
                                                    bqk_t[:, ft:ft + 1])
                        k_ps = psp.tile([128, 512], F32, tag="mm")
                        for c in range(8):
                            nc.tensor.matmul(k_ps[:], wk_t[c][:, ft * 128:(ft + 1) * 128],
                                             xt_t[c][nt][:], start=(c == 0), stop=(c == 7))
                        nc.vector.tensor_scalar_add(kS[ft][nt][:], k_ps[:],
                                                    bqk_t[:, 4 + ft:5 + ft])
                    for t2 in range(4):
                        tt = nt * 4 + t2
                        v_ps = psp.tile([128, 512], F32, tag="mm")
                        for c in range(8):
                            nc.tensor.matmul(v_ps[:], xt_t[c][nt][:, t2 * 128:(t2 + 1) * 128],
                                             wv_t[c][:], start=(c == 0), stop=(c == 7))
                        vv = vS[tt][:].rearrange("p (h c) -> p h c", c=65)
                        nc.vector.tensor_add(vv[:, :, 0:64],
                                             v_ps[:].rearrange("p (h c) -> p h c", c=64),
                                             bvb_t[:].rearrange("p (h c) -> p h c", c=64))

                def P2(qt, fills=None, sc_fills=None):
                    # ---------------- P2: attention for query chunk qt ----------------
                    nblk = 4 * qt + 4
                    for pj in range(4):
                        y_ps = [psp.tile([65, 512], F32, tag=f"yps{h}", bufs=1, name=f"yps{h}")
                                for h in range(2)]
                        for sc in range(nblk):
                            r = sc - 4 * qt
                            qlo = 128 * r if r > 0 else 0
                            ntk, kk = sc // 4, sc % 4
                            s_ps = psp.tile([128, 1024], F32, tag="sps")
                            nc.tensor.matmul(s_ps[:, qlo:512],
                                             kS[pj][ntk][0:64, kk * 128:(kk + 1) * 128],
                                             qS[pj][qt][0:64, qlo:512],
                                             start=True, stop=True, tile_position=(0, 0))
                            nc.tensor.matmul(s_ps[:, 512 + qlo:1024],
                                             kS[pj][ntk][64:128, kk * 128:(kk + 1) * 128],
                                             qS[pj][qt][64:128, qlo:512],
                                             start=True, stop=True, tile_position=(64, 0))
                            e_t = ep.tile([128, 1024], BF16, tag="e")
                            if r < 0:
                                nc.scalar.activation(e_t[:], s_ps[:], AFT.Exp,
                                                     scale=0.125)
                            else:
                                sv = s_ps[:].rearrange("p (g n) -> p g n",
                                                       g=2)[:, :, qlo:512]
                                ev = e_t[:].rearrange("p (g n) -> p g n",
                                                      g=2)[:, :, qlo:512]
                                nc.scalar.activation(ev, sv, AFT.Exp, scale=0.125)
                            if r >= 0:
                                em = e_t[:].rearrange("p (g n) -> p g n", g=2)[:, :, qlo:qlo + 128]
                                nc.vector.tensor_mul(
                                    em, em, tri2_t[:].rearrange("p (g n) -> p g n", g=2))
                            for h in range(2):
                                hc = 130 * pj + 65 * h
                                nc.tensor.matmul(y_ps[h][:, qlo:512],
                                                 vS[sc][:, hc:hc + 65],
                                                 e_t[:, 512 * h + qlo:512 * h + 512],
                                                 start=(sc == 0), stop=(sc == nblk - 1))
                            if sc_fills:
                                f = sc_fills.pop((pj, sc), None)
                                if f is not None:
                                    f()
                        for h in range(2):
                            y_sb = smallp.tile([65, 512], F32, tag="ysb", bufs=6)
                            nc.vector.tensor_copy(y_sb[:], y_ps[h][:])
                            den = smallp.tile([1, 512], F32, tag="den", bufs=4)
                            nc.sync.dma_start(den[:], y_sb[64:65, :])
                            rec = smallp.tile([1, 512], F32, tag="rec", bufs=4)
                            nc.vector.reciprocal_approx_fast(rec[:], den[:])
                            rb = smallp.tile([64, 512], F32, tag="rb", bufs=4)
                            nc.gpsimd.partition_broadcast(rb[:], rec[:])
                            nc.vector.tensor_mul(yS[pj][qt][64 * h:64 * h + 64, :],
                                                 y_sb[0:64, :], rb[:])
                        if fills:
                            for f in fills.get(pj, ()):
                                f()

                def P3_group(qt, t2, of):
                    tt = 4 * qt + t2
                    o_ps = psp.tile([128, 512], F32, tag="mm")
                    for i, pj in enumerate((3, 0, 1, 2)):
                        nc.tensor.matmul(o_ps[:],
                                         yS[pj][qt][:, t2 * 128:(t2 + 1) * 128],
                                         wp_t[pj][:, of * 512:(of + 1) * 512],
                                         start=(i == 0), stop=(i == 3))
                    o_t = smallp.tile([128, 512], F32, tag="osb", bufs=3)
                    nc.vector.tensor_add(o_t[:], o_ps[:],
                                         wpb_t[:, of * 512:(of + 1) * 512])
                    nc.sync.dma_start(out_d[tt * 128:(tt + 1) * 128,
                                            of * 512:(of + 1) * 512], o_t[:])

                def P3_fills(qt):
                    gs = [(t2, of) for t2 in range(4) for of in range(2)]
                    return {pj: [lambda t2=t2, of=of: P3_group(qt, t2, of)
                                 for (t2, of) in gs[2 * pj:2 * pj + 2]]
                            for pj in range(4)}

                # pass-A/pass-B split of the last projection: partials over
                # head-pairs 0-2 (+bias) computed during P2(3); only one
                # matmul per group remains after the last normalization.
                parts = [smallp.tile([128, 512], F32, tag=f"part{g}",
                                     bufs=1, name=f"part{g}") for g in range(8)]

                def P3a_group(qt, t2, of):
                    o_ps = psp.tile([128, 512], F32, tag="mm")
                    for i, pj in enumerate((0, 1, 2)):
                        nc.tensor.matmul(o_ps[:],
                                         yS[pj][qt][:, t2 * 128:(t2 + 1) * 128],
                                         wp_t[pj][:, of * 512:(of + 1) * 512],
                                         start=(i == 0), stop=(i == 2))
                    nc.vector.tensor_add(parts[t2 * 2 + of][:], o_ps[:],
                                         wpb_t[:, of * 512:(of + 1) * 512])

                def P3b(qt):
                    for t2 in range(4):
                        tt = 4 * qt + t2
                        for of in range(2):
                            o_ps = psp.tile([128, 512], F32, tag="mm")
                            nc.tensor.matmul(o_ps[:],
                                             yS[3][qt][:, t2 * 128:(t2 + 1) * 128],
                                             wp_t[3][:, of * 512:(of + 1) * 512],
                                             start=True, stop=True)
                            o_t = smallp.tile([128, 512], F32, tag="osb", bufs=3)
                            nc.vector.tensor_add(o_t[:], o_ps[:],
                                                 parts[t2 * 2 + of][:])
                            nc.sync.dma_start(out_d[tt * 128:(tt + 1) * 128,
                                                    of * 512:(of + 1) * 512], o_t[:])

                # P1(nt+1) is ordered before P2(nt+1); P3(qt-1) groups are
                # spread across P2(qt)'s head-pair boundaries to fill its
                # exp-bound stretches; P3(3) is split so only one matmul per
                # group trails the last normalization.
                P1(0)
                P2(0)
                P1(1)
                P2(1, fills=P3_fills(0))
                P1(2)
                P2(2, fills=P3_fills(1))
                P1(3)
                f2 = P3_fills(2)
                scf = {(3, 2 + 3 * g): (lambda g=g: P3a_group(3, g // 2, g % 2))
                       for g in range(5)}
                f3 = {2: f2[2] + f2[3] + [lambda: P3a_group(3, 2, 1),
                                          lambda: P3a_group(3, 3, 0),
                                          lambda: P3a_group(3, 3, 1)]}
                P2(3, fills={**{pj: f2[pj] for pj in (0, 1)}, **f3},
                   sc_fills=scf)
                P3b(3)

    if not nc.is_finalized():
        nc.finalize()
    return nc


def _get_nc():
    if "nc" not in _CACHE:
        _CACHE["nc"] = build()
    return _CACHE["nc"]


def _tri2():
    k = np.arange(128)[:, None]
    q = np.arange(128)[None, :]
    t = np.where(k <= q, 1.0, 0.0).astype(ml_dtypes.bfloat16)
    return np.concatenate([t, t], axis=1)


def kernel(x, w_attn, b_attn, w_proj, b_proj, _trace=False, _trace_kwargs=None):
    x = np.asarray(x, dtype=np.float32)
    w_attn = np.asarray(w_attn, dtype=np.float32)
    b_attn = np.asarray(b_attn, dtype=np.float32)
    w_proj = np.asarray(w_proj, dtype=np.float32)
    b_proj = np.asarray(b_proj, dtype=np.float32)

    bf = ml_dtypes.bfloat16
    tri2 = _tri2()
    in_maps = []
    for core in range(8):
        b, hg = core // 2, core % 2
        cs = hg * 512
        bq = b_attn[cs:cs + 512]
        bk = b_attn[C + cs:C + cs + 512]
        bv = b_attn[2 * C + cs:2 * C + cs + 512]
        bqk = np.concatenate([bq.reshape(4, 128).T, bk.reshape(4, 128).T],
                             axis=1).astype(np.float32)
        wpb = b_proj if hg == 0 else np.zeros_like(b_proj)
        in_maps.append({
            "xt": np.ascontiguousarray(
                x[b].T.reshape(8, 128, 4, 512).transpose(0, 2, 1, 3)
                .reshape(4096, 512)).astype(bf),
            "wq": np.ascontiguousarray(w_attn[:, cs:cs + 512]).astype(bf),
            "wk": np.ascontiguousarray(w_attn[:, C + cs:C + cs + 512]).astype(bf),
            "wv": np.ascontiguousarray(w_attn[:, 2 * C + cs:2 * C + cs + 512]).astype(bf),
            "bqk": bqk,
            "bvb": np.ascontiguousarray(
                np.broadcast_to(bv[None, :], (128, 512))).astype(np.float32),
            "tri2": tri2,
            "wp": np.ascontiguousarray(w_proj[cs:cs + 512, :]).astype(bf),
            "wpb": np.ascontiguousarray(
                np.broadcast_to(wpb[None, :], (128, C))).astype(np.float32),
        })

    kw = {}
    if _trace:
        kw["trace"] = True
        if _trace_kwargs:
            kw.update(_trace_kwargs)
    res = run_bass_kernel_spmd(_get_nc(), in_maps, list(range(8)), **kw)
    _CACHE["last_results"] = res
    outs = [res.results[c]["out"] for c in range(8)]
    y = np.stack([outs[2 * b] + outs[2 * b + 1] for b in range(B)])
    return y.astype(np.float32)
